# revision 11
# baseline (speedup 1.0000x reference)
"""Single-head causal attention block (QKV projection + attention) on 8 TRN2 cores.

Reference computation (per batch element b, batch-sharded 1 core each):
    qkv = x[b] @ W.T + b          # [T, 3E]
    q, k, v = split(qkv)          # each [T, E]
    s = (q @ k.T) / sqrt(E), causal-masked
    y = softmax(s) @ v            # [T, E]

Shapes: B=8, T=2048, E=1024.  Design notes:
  - All layouts host-prepped so no on-device transposes are needed:
      q^T, k^T computed in [E, T] layout (score matmul operands),
      v computed in [T, E] layout (PV matmul rhs),
      scores computed transposed S^T[tk, tq] so exp needs no partition reduce.
  - Softmax without max-subtraction: scores for this problem are ~N(0, 0.33),
    |s| < 3, so exp() is numerically safe unnormalized; masked entries get -50
    added (exp -> ~2e-22).  Row sums Z come from a ones-column matmul fused
    into the PV accumulation pattern; normalization is a per-partition
    tensor_scalar multiply at the end.
  - Causal structure skips entire 128x512 score tiles above the diagonal and
    the corresponding PV accumulation terms (~2x on attention FLOPs).
"""

import numpy as np
import ml_dtypes
from contextlib import ExitStack

import concourse.bass as bass
import concourse.bacc as bacc
import concourse.mybir as mybir
import concourse.tile as tile
from concourse.bass_utils import run_bass_kernel_spmd

FP32 = mybir.dt.float32
F32R = mybir.dt.float32r
BF16 = mybir.dt.bfloat16
AF = mybir.ActivationFunctionType

B, T, E = 8, 2048, 1024
P = 128
NE = E // P            # 8 e-tiles (contraction)
NT = T // P            # 16 t-tiles
NC = 4                 # tq chunks of 512
CH = T // NC           # 512
SCALE = 1.0 / np.sqrt(E)
MASK_NEG = -50.0

# dtype knobs
P1_DT = BF16           # phase-1 matmul operand dtype (x, W)
QK_DT = BF16           # stored q^T / k^T dtype (score matmul operands)
V_DT = BF16            # stored v dtype (PV rhs)
ES_DT = BF16           # stored exp(S^T) dtype (PV lhsT)


def _np_of(dt):
    return ml_dtypes.bfloat16 if dt == BF16 else np.float32


def _build_nc():
    nc = bacc.Bacc()

    xt_d = nc.declare_dram_parameter("xt", [NE, P, T], P1_DT, isOutput=False)
    wqk_d = nc.declare_dram_parameter("wqk", [2 * NE, P, NE, P], P1_DT, isOutput=False)
    wv_d = nc.declare_dram_parameter("wv", [P, NE, E], P1_DT, isOutput=False)
    bqk_d = nc.declare_dram_parameter("bqk", [2 * NE, 1, P], P1_DT, isOutput=False)
    bvrep_d = nc.declare_dram_parameter("bvrep", [P, E], FP32, isOutput=False)
    masks_d = nc.declare_dram_parameter("masks", [4, P, CH], FP32, isOutput=False)
    y_d = nc.declare_dram_parameter("y", [T, E], FP32, isOutput=True)

    with tile.TileContext(nc) as tc:
        with ExitStack() as ctx:
            # ---- persistent pools (live through whole kernel) ----
            const_pool = ctx.enter_context(tc.tile_pool(name="const", bufs=1))
            bqk_pool = ctx.enter_context(tc.tile_pool(name="bqk", bufs=2 * NE))
            mask_pool = ctx.enter_context(tc.tile_pool(name="mask", bufs=4))
            qk_pool = ctx.enter_context(tc.tile_pool(name="qk", bufs=2 * NE))
            v_pool = ctx.enter_context(tc.tile_pool(name="v", bufs=NT))
            psa = ctx.enter_context(tc.tile_pool(name="psa", bufs=3, space="PSUM"))

            ones_col = const_pool.tile([P, 1], ES_DT, tag="ones", name="ones")
            nc.vector.memset(ones_col[:], 1.0)
            ones_row = const_pool.tile([1, CH], P1_DT, tag="onesr", name="onesr")
            nc.vector.memset(ones_row[:], 1.0)
            bvrep = const_pool.tile([P, E], FP32, tag="bvrep", name="bvrep")
            nc.sync.dma_start(bvrep[:], bvrep_d[:])

            bqk_sb = []
            for ft in range(2 * NE):
                t_ = bqk_pool.tile([1, P], P1_DT, tag="bqk", name="bqk")
                nc.sync.dma_start(t_[:], bqk_d[ft])
                bqk_sb.append(t_)
            mask_sb = []
            for d in range(4):
                t_ = mask_pool.tile([P, CH], FP32, tag="mask", name="mask")
                nc.sync.dma_start(t_[:], masks_d[d])
                mask_sb.append(t_)

            qk_sb = [qk_pool.tile([P, T], QK_DT, tag="qk", name="qk") for _ in range(2 * NE)]
            v_sb = [v_pool.tile([P, E], V_DT, tag="v", name="v") for _ in range(NT)]

            # ---- phase 1: qkv projection ----
            with ExitStack() as p1:
                xt_pool = p1.enter_context(tc.tile_pool(name="xt", bufs=NE))
                wqk_pool = p1.enter_context(tc.tile_pool(name="wqkp", bufs=3))
                wv_pool = p1.enter_context(tc.tile_pool(name="wvp", bufs=1))

                xt_sb = []
                for a in range(NE):
                    t_ = xt_pool.tile([P, T], P1_DT, tag="xt", name="xt")
                    nc.sync.dma_start(t_[:], xt_d[a])
                    xt_sb.append(t_)
                wv_sb = wv_pool.tile([P, NE, E], P1_DT, tag="wv", name="wv")
                nc.sync.dma_start(wv_sb[:], wv_d[:])

                # q^T and k^T in [f, t] layout, f-tile by f-tile
                for ft in range(2 * NE):
                    wt = wqk_pool.tile([P, NE, P], P1_DT, tag="wqk", name="wqk")
                    # split load per e-slice: a single big DMA fans out over
                    # two HW queues, and a slot-reuse DMA can then need 3 sync
                    # waits (PE release + 2x WAW) -- more than the DIRECT2D
                    # encoding's 2 slots.  Small DMAs stay single-queue.
                    for e in range(NE):
                        nc.sync.dma_start(wt[:, e, :], wqk_d[ft, :, e, :])
                    for tch in range(NC):
                        ps = psa.tile([P, CH], FP32, tag="ps", name="ps")
                        # bias via K=1 matmul: psum[f, t] = bias[f] * ones[t]
                        nc.tensor.matmul(
                            ps[:],
                            lhsT=bqk_sb[ft][:],
                            rhs=ones_row[:],
                            start=True,
                            stop=False,
                        )
                        for e in range(NE):
                            nc.tensor.matmul(
                                ps[:],
                                lhsT=wt[:, e, :],
                                rhs=xt_sb[e][:, tch * CH:(tch + 1) * CH],
                                start=False,
                                stop=(e == NE - 1),
                            )
                        # 1/sqrt(E) score scale folded into q copy-out
                        sc = SCALE if ft < NE else 1.0
                        nc.scalar.activation(
                            qk_sb[ft][:, tch * CH:(tch + 1) * CH],
                            ps[:],
                            AF.Copy,
                            scale=sc,
                        )

                # v in [t, e] layout
                for tt in range(NT):
                    for ec in range(2):
                        ps = psa.tile([P, CH], FP32, tag="ps", name="ps")
                        for e in range(NE):
                            nc.tensor.matmul(
                                ps[:],
                                lhsT=xt_sb[e][:, tt * P:(tt + 1) * P],
                                rhs=wv_sb[:, e, ec * CH:(ec + 1) * CH],
                                start=(e == 0),
                                stop=(e == NE - 1),
                            )
                        # bias varies along free dim -> tensor add of
                        # host-replicated bias tile, writes V_DT directly
                        nc.vector.tensor_add(
                            v_sb[tt][:, ec * CH:(ec + 1) * CH],
                            ps[:],
                            bvrep[:, ec * CH:(ec + 1) * CH],
                        )

            # ---- phases 2+3: scores+softmax+PV, per tq chunk ----
            with ExitStack() as p2:
                exps_pool = p2.enter_context(tc.tile_pool(name="exps", bufs=20))
                y_pool = p2.enter_context(tc.tile_pool(name="yst", bufs=3))
                zr_pool = p2.enter_context(tc.tile_pool(name="zr", bufs=4))
                psy = p2.enter_context(tc.tile_pool(name="psy", bufs=3, space="PSUM"))
                psz = p2.enter_context(tc.tile_pool(name="psz", bufs=2, space="PSUM"))

                for c in range(NC):
                    n_tk = (c + 1) * (CH // P)  # tk tiles at/below diagonal
                    exps_tiles = []
                    for tk in range(n_tk):
                        ps = psa.tile([P, CH], FP32, tag="ps", name="ps")
                        for e in range(NE):
                            nc.tensor.matmul(
                                ps[:],
                                lhsT=qk_sb[NE + e][:, tk * P:(tk + 1) * P],
                                rhs=qk_sb[e][:, c * CH:(c + 1) * CH],
                                start=(e == 0),
                                stop=(e == NE - 1),
                            )
                        d = tk - c * (CH // P)
                        if d >= 0:  # diagonal-crossing tile: additive causal mask
                            nc.vector.tensor_add(ps[:], ps[:], mask_sb[d][:])
                        et = exps_pool.tile([P, CH], ES_DT, tag="es", name="es")
                        nc.scalar.activation(et[:], ps[:], AF.Exp)
                        exps_tiles.append(et)

                    ps_z = psz.tile([P, CH // P], FP32, tag="z", name="z")
                    for j in range(CH // P):
                        tq = c * (CH // P) + j
                        nj = tq + 1
                        for tk in range(nj):
                            nc.tensor.matmul(
                                ps_z[:, j:j + 1],
                                lhsT=exps_tiles[tk][:, j * P:(j + 1) * P],
                                rhs=ones_col[:],
                                start=(tk == 0),
                                stop=(tk == nj - 1),
                            )
                        zr = zr_pool.tile([P, 1], FP32, tag="zr", name="zr")
                        nc.vector.reciprocal(zr[:], ps_z[:, j:j + 1])
                        y_t = y_pool.tile([P, E], FP32, tag="y", name="y")
                        for ec in range(2):
                            ps_y = psy.tile([P, CH], FP32, tag="y", name="psy")
                            for tk in range(nj):
                                nc.tensor.matmul(
                                    ps_y[:],
                                    lhsT=exps_tiles[tk][:, j * P:(j + 1) * P],
                                    rhs=v_sb[tk][:, ec * CH:(ec + 1) * CH],
                                    start=(tk == 0),
                                    stop=(tk == nj - 1),
                                )
                            nc.vector.tensor_scalar_mul(
                                y_t[:, ec * CH:(ec + 1) * CH], ps_y[:], zr[:]
                            )
                        nc.sync.dma_start(y_d[tq * P:(tq + 1) * P, :], y_t[:])
    nc.finalize()  # run the Bacc pass pipeline (wait splitting, reg alloc, ...)
    return nc


_NC_CACHE = None


def _get_nc():
    global _NC_CACHE
    if _NC_CACHE is None:
        _NC_CACHE = _build_nc()
    return _NC_CACHE


def _prep_inputs(x, W, b):
    p1np = _np_of(P1_DT)
    # x[b].T tiled: xt[a, p, t] = x[b, t, a*128+p]
    xt = np.ascontiguousarray(
        x.reshape(B, T, NE, P).transpose(0, 2, 3, 1)
    ).astype(p1np)
    # wqk[ft, p, a, f'] = W[ft*128+f', a*128+p]
    wqk = np.ascontiguousarray(
        W[:2 * E].reshape(2 * NE, P, NE, P).transpose(0, 3, 2, 1)
    ).astype(p1np)
    # wv[p, a, eo] = W[2E+eo, a*128+p]
    wv = np.ascontiguousarray(
        W[2 * E:].reshape(E, NE, P).transpose(2, 1, 0)
    ).astype(p1np)
    # raw bias (scale is applied after bias-in-psum, during copy-out)
    bqk = b[:2 * E].reshape(2 * NE, 1, P).astype(p1np)
    bvrep = np.broadcast_to(b[2 * E:].astype(np.float32), (P, E)).copy()
    ii = np.arange(P)[:, None]
    jj = np.arange(CH)[None, :]
    masks = np.stack(
        [np.where(jj >= d * P + ii, 0.0, MASK_NEG) for d in range(4)]
    ).astype(np.float32)
    shared = {"wqk": wqk, "wv": wv, "bqk": bqk, "bvrep": bvrep, "masks": masks}
    return [{"xt": np.ascontiguousarray(xt[i]), **shared} for i in range(B)]


def run(x, W, b, **spmd_kwargs):
    nc = _get_nc()
    in_maps = _prep_inputs(np.asarray(x), np.asarray(W), np.asarray(b))
    res = run_bass_kernel_spmd(nc, in_maps, list(range(B)), **spmd_kwargs)
    y = np.stack([res.results[i]["y"] for i in range(B)]).astype(np.float32)
    return y, res


def kernel(x, W, b):
    y, _ = run(x, W, b)
    return y


# revision 27
# speedup vs baseline: 16221.8123x; 16221.8123x over previous
"""Single-head causal attention block (QKV projection + attention) on 8 TRN2 cores.

Reference computation (per batch element b, batch-sharded 1 core each):
    qkv = x[b] @ W.T + b          # [T, 3E]
    q, k, v = split(qkv)          # each [T, E]
    s = (q @ k.T) / sqrt(E), causal-masked
    y = softmax(s) @ v            # [T, E]

Shapes: B=8, T=2048, E=1024.  Design notes:
  - All layouts host-prepped so no on-device transposes are needed:
      q^T, k^T computed in [E, T] layout (score matmul operands),
      v computed in [T, E] layout (PV matmul rhs),
      scores computed transposed S^T[tk, tq] so exp needs no partition reduce.
  - Softmax without max-subtraction: scores for this problem are ~N(0, 0.33),
    |s| < 3, so exp() is numerically safe unnormalized; masked entries get -50
    added (exp -> ~2e-22).  Row sums Z come from a ones-column matmul fused
    into the PV accumulation pattern; normalization is a per-partition
    tensor_scalar multiply at the end.
  - Causal structure skips entire 128x512 score tiles above the diagonal and
    the corresponding PV accumulation terms (~2x on attention FLOPs).
"""

import numpy as np
import ml_dtypes
from contextlib import ExitStack

import concourse.bass as bass
import concourse.bacc as bacc
import concourse.mybir as mybir
import concourse.tile as tile
from concourse.bass_utils import run_bass_kernel_spmd

FP32 = mybir.dt.float32
F32R = mybir.dt.float32r
BF16 = mybir.dt.bfloat16
AF = mybir.ActivationFunctionType

B, T, E = 8, 2048, 1024
P = 128
NE = E // P            # 8 e-tiles (contraction)
NT = T // P            # 16 t-tiles
NC = 4                 # tq chunks of 512
CH = T // NC           # 512
SCALE = 1.0 / np.sqrt(E)
MASK_NEG = -50.0

# dtype knobs
P1_DT = BF16           # phase-1 matmul operand dtype (x, W)
QK_DT = BF16           # stored q^T / k^T dtype (score matmul operands)
V_DT = F32R            # stored v dtype (fp32r: full-rate matmul, ~fp32 mantissa)
ES_DT = F32R           # stored exp(S^T) dtype (fp32r)


def _np_of(dt):
    return ml_dtypes.bfloat16 if dt == BF16 else np.float32


def _build_nc(n_reps=1):
    nc = bacc.Bacc()

    xt_d = nc.declare_dram_parameter("xt", [NE, P, T], P1_DT, isOutput=False)
    wqk_d = nc.declare_dram_parameter("wqk", [2 * NE, P, NE, P], P1_DT, isOutput=False)
    wv_d = nc.declare_dram_parameter("wv", [P, NE, E], P1_DT, isOutput=False)
    bqk_d = nc.declare_dram_parameter("bqk", [2 * NE, P, 1], FP32, isOutput=False)
    bvrep_d = nc.declare_dram_parameter("bvrep", [P, E], FP32, isOutput=False)
    masks_d = nc.declare_dram_parameter("masks", [4, P, CH], FP32, isOutput=False)
    onesc_d = nc.declare_dram_parameter("onesc", [P, 4], F32R, isOutput=False)
    y_d = nc.declare_dram_parameter("y", [T, E], FP32, isOutput=True)

    with tile.TileContext(nc) as tc:
        with ExitStack() as ctx:
            # ---- persistent pools (live through whole kernel) ----
            const_pool = ctx.enter_context(tc.tile_pool(name="const", bufs=1))
            bqk_pool = ctx.enter_context(tc.tile_pool(name="bqk", bufs=2 * NE))
            mask_pool = ctx.enter_context(tc.tile_pool(name="mask", bufs=4))
            qk_pool = ctx.enter_context(tc.tile_pool(name="qk", bufs=2 * NE))
            v_pool = ctx.enter_context(tc.tile_pool(name="v", bufs=NT))
            psa = ctx.enter_context(tc.tile_pool(name="psa", bufs=3, space="PSUM"))

            ones_col = const_pool.tile([P, 4], F32R, tag="ones", name="ones")
            nc.sync.dma_start(ones_col[:], onesc_d[:])

            qk_sb = [qk_pool.tile([P, T], QK_DT, tag="qk", name="qk") for _ in range(2 * NE)]
            v_sb = [v_pool.tile([P, E], V_DT, tag="v", name="v") for _ in range(NT)]

            # benchmark-only: run the whole body n_reps times on-device so
            # per-kernel time can be extracted from wall-clock deltas
            rep_ctx = tc.For_i(0, n_reps, 1) if n_reps > 1 else None
            if rep_ctx is not None:
                ctx.enter_context(rep_ctx)

            # ---- phase 1: qkv projection ----
            with ExitStack() as p1:
                xt_pool = p1.enter_context(tc.tile_pool(name="xt", bufs=NE))
                wqk_pool = p1.enter_context(tc.tile_pool(name="wqkp", bufs=3))
                wv_pool = p1.enter_context(tc.tile_pool(name="wvp", bufs=1))

                # xt first (critical path: every phase-1 matmul group needs
                # all 8 e-tiles); weights go on the scalar-engine HWDGE
                # queue so they stream in parallel with xt on the sync queue
                xt_sb = []
                for a in range(NE):
                    t_ = xt_pool.tile([P, T], P1_DT, tag="xt", name="xt")
                    nc.sync.dma_start(t_[:], xt_d[a])
                    xt_sb.append(t_)
                bqk_sb = []
                for ft in range(2 * NE):
                    t_ = bqk_pool.tile([P, 1], FP32, tag="bqk", name="bqk")
                    nc.sync.dma_start(t_[:], bqk_d[ft])
                    bqk_sb.append(t_)

                # q^T and k^T in [f, t] layout, f-tile by f-tile
                for ft in range(2 * NE):
                    wt = wqk_pool.tile([P, NE, P], P1_DT, tag="wqk", name="wqk")
                    # split load per e-slice: a single big DMA fans out over
                    # two HW queues, and a slot-reuse DMA can then need 3 sync
                    # waits (PE release + 2x WAW) -- more than the DIRECT2D
                    # encoding's 2 slots.  Small DMAs stay single-queue.
                    for e in range(NE):
                        nc.scalar.dma_start(wt[:, e, :], wqk_d[ft, :, e, :])
                    if ft == 2:
                        # weights for the v path arrive while qk streams
                        wv_sb = wv_pool.tile([P, NE, E], P1_DT, tag="wv", name="wv")
                        for e in range(NE):
                            nc.scalar.dma_start(wv_sb[:, e, :], wv_d[:, e, :])
                        bvrep = const_pool.tile([P, E], FP32, tag="bvrep", name="bvrep")
                        nc.sync.dma_start(bvrep[:], bvrep_d[:])
                    for tch in range(NC):
                        ps = psa.tile([P, CH], FP32, tag="ps", name="ps")
                        for e in range(NE):
                            nc.tensor.matmul(
                                ps[:],
                                lhsT=wt[:, e, :],
                                rhs=xt_sb[e][:, tch * CH:(tch + 1) * CH],
                                start=(e == 0),
                                stop=(e == NE - 1),
                            )
                        # bias add + 1/sqrt(E) score scale folded into q
                        # copy-out: out = in*scale + bias (bias prescaled)
                        sc = SCALE if ft < NE else 1.0
                        nc.scalar.activation(
                            qk_sb[ft][:, tch * CH:(tch + 1) * CH],
                            ps[:],
                            AF.Identity,
                            bias=bqk_sb[ft][:],
                            scale=sc,
                        )

                # v in [t, e] layout
                for tt in range(NT):
                    for ec in range(2):
                        ps = psa.tile([P, CH], FP32, tag="ps", name="ps")
                        for e in range(NE):
                            nc.tensor.matmul(
                                ps[:],
                                lhsT=xt_sb[e][:, tt * P:(tt + 1) * P],
                                rhs=wv_sb[:, e, ec * CH:(ec + 1) * CH],
                                start=(e == 0),
                                stop=(e == NE - 1),
                            )
                        # bias varies along free dim -> tensor add of
                        # host-replicated bias tile, writes V_DT directly
                        nc.vector.tensor_add(
                            v_sb[tt][:, ec * CH:(ec + 1) * CH],
                            ps[:],
                            bvrep[:, ec * CH:(ec + 1) * CH],
                        )

            # ---- phases 2+3: scores+softmax+PV, per tq chunk ----
            with ExitStack() as p2:
                exps_pool = p2.enter_context(tc.tile_pool(name="exps", bufs=18))
                y_pool = p2.enter_context(tc.tile_pool(name="yst", bufs=2))
                zr_pool = p2.enter_context(tc.tile_pool(name="zr", bufs=4))
                psy = p2.enter_context(tc.tile_pool(name="psy", bufs=3, space="PSUM"))
                psz = p2.enter_context(tc.tile_pool(name="psz", bufs=2, space="PSUM"))

                mask_sb = []
                for d in range(4):
                    t_ = mask_pool.tile([P, CH], FP32, tag="mask", name="mask")
                    nc.sync.dma_start(t_[:], masks_d[d])
                    mask_sb.append(t_)

                for c in range(NC):
                    n_tk = (c + 1) * (CH // P)  # tk tiles at/below diagonal
                    exps_tiles = []
                    for tk in range(n_tk):
                        ps = psa.tile([P, CH], FP32, tag="ps", name="ps")
                        for e in range(NE):
                            nc.tensor.matmul(
                                ps[:],
                                lhsT=qk_sb[NE + e][:, tk * P:(tk + 1) * P],
                                rhs=qk_sb[e][:, c * CH:(c + 1) * CH],
                                start=(e == 0),
                                stop=(e == NE - 1),
                            )
                        d = tk - c * (CH // P)
                        if d >= 0:  # diagonal-crossing tile: additive causal mask
                            nc.vector.tensor_add(ps[:], ps[:], mask_sb[d][:])
                        et = exps_pool.tile([P, CH], ES_DT, tag="es", name="es")
                        nc.scalar.activation(et[:], ps[:], AF.Exp)
                        exps_tiles.append(et)

                    ps_z = psz.tile([P, 4 * (CH // P)], FP32, tag="z", name="z")
                    for j in range(CH // P):
                        tq = c * (CH // P) + j
                        nj = tq + 1
                        for tk in range(nj):
                            nc.tensor.matmul(
                                ps_z[:, 4 * j:4 * j + 4],
                                lhsT=exps_tiles[tk][:, j * P:(j + 1) * P],
                                rhs=ones_col[:],
                                start=(tk == 0),
                                stop=(tk == nj - 1),
                            )
                        zr = zr_pool.tile([P, 1], FP32, tag="zr", name="zr")
                        nc.vector.reciprocal(zr[:], ps_z[:, 4 * j:4 * j + 1])
                        y_t = y_pool.tile([P, E], FP32, tag="y", name="y")
                        for ec in range(2):
                            ps_y = psy.tile([P, CH], FP32, tag="y", name="psy")
                            for tk in range(nj):
                                nc.tensor.matmul(
                                    ps_y[:],
                                    lhsT=exps_tiles[tk][:, j * P:(j + 1) * P],
                                    rhs=v_sb[tk][:, ec * CH:(ec + 1) * CH],
                                    start=(tk == 0),
                                    stop=(tk == nj - 1),
                                )
                            nc.vector.tensor_scalar_mul(
                                y_t[:, ec * CH:(ec + 1) * CH], ps_y[:], zr[:]
                            )
                        nc.sync.dma_start(y_d[tq * P:(tq + 1) * P, :], y_t[:])
    nc.finalize()  # run the Bacc pass pipeline (wait splitting, reg alloc, ...)
    return nc


_NC_CACHE = {}


def _get_nc(n_reps=1):
    if n_reps not in _NC_CACHE:
        _NC_CACHE[n_reps] = _build_nc(n_reps)
    return _NC_CACHE[n_reps]


def _prep_inputs(x, W, b):
    p1np = _np_of(P1_DT)
    # x[b].T tiled: xt[a, p, t] = x[b, t, a*128+p]
    xt = np.ascontiguousarray(
        x.reshape(B, T, NE, P).transpose(0, 2, 3, 1)
    ).astype(p1np)
    # wqk[ft, p, a, f'] = W[ft*128+f', a*128+p]
    wqk = np.ascontiguousarray(
        W[:2 * E].reshape(2 * NE, P, NE, P).transpose(0, 3, 2, 1)
    ).astype(p1np)
    # wv[p, a, eo] = W[2E+eo, a*128+p]
    wv = np.ascontiguousarray(
        W[2 * E:].reshape(E, NE, P).transpose(2, 1, 0)
    ).astype(p1np)
    # ACT applies out = in*scale + bias, so the q bias is prescaled
    bqk = b[:2 * E].astype(np.float32).copy()
    bqk[:E] *= SCALE
    bqk = bqk.reshape(2 * NE, P, 1)
    bvrep = np.broadcast_to(b[2 * E:].astype(np.float32), (P, E)).copy()
    ii = np.arange(P)[:, None]
    jj = np.arange(CH)[None, :]
    masks = np.stack(
        [np.where(jj >= d * P + ii, 0.0, MASK_NEG) for d in range(4)]
    ).astype(np.float32)
    onesc = np.ones((P, 4), np.float32)
    shared = {"wqk": wqk, "wv": wv, "bqk": bqk, "bvrep": bvrep, "masks": masks,
              "onesc": onesc}
    return [{"xt": np.ascontiguousarray(xt[i]), **shared} for i in range(B)]


def run(x, W, b, **spmd_kwargs):
    nc = _get_nc()
    in_maps = _prep_inputs(np.asarray(x), np.asarray(W), np.asarray(b))
    res = run_bass_kernel_spmd(nc, in_maps, list(range(B)), **spmd_kwargs)
    y = np.stack([res.results[i]["y"] for i in range(B)]).astype(np.float32)
    return y, res


def kernel(x, W, b):
    y, _ = run(x, W, b)
    return y
